# revision 3
# baseline (speedup 1.0000x reference)
"""Trainium2 Bass kernel for nn_Loss_fn_21852793602431 (DETR-style loss).

Strategy (data-parallel over batch B=64, 8 cores x 8 batches):
  - host: preprocess per-box quantities (f32, mirroring the reference
    formula order) into per-core device inputs
  - device (SPMD x8): build the transposed DIoU pair-cost slab
    slabT[b, m, n] = dist/diag - iou_e  for the core's 8 batches, plus the
    core-partial L1 cost matrix sum_{b_loc,c} |pred - tgt| (transposed)
  - host: label (BCE) cost via matmul, sum the 8 L1 partials, add, run
    scipy linear_sum_assignment per batch, compute the final scalar losses
"""

import sys

if "/opt/trn_rl_repo" not in sys.path:
    sys.path.insert(0, "/opt/trn_rl_repo")

import numpy as np

B, N, M = 64, 256, 256
NCORES = 8
BL = B // NCORES
NQ = 11
EPS32 = np.float32(1e-7)

_bass_module = None


def _build_bass():
    import concourse.bacc as bacc
    from concourse import mybir, tile
    from contextlib import ExitStack

    f32 = mybir.dt.float32
    AF = mybir.ActivationFunctionType
    OP = mybir.AluOpType

    nc = bacc.Bacc("TRN2", target_bir_lowering=False, debug=False,
                   num_devices=NCORES)
    predq = nc.dram_tensor("predq", [BL, NQ * N], f32, kind="ExternalInput").ap()
    tgtq = nc.dram_tensor("tgtq", [BL, M, NQ], f32, kind="ExternalInput").ap()
    slab = nc.dram_tensor("slab", [BL, M, N], f32, kind="ExternalOutput").ap()
    l1p = nc.dram_tensor("l1p", [M, N], f32, kind="ExternalOutput").ap()

    with tile.TileContext(nc) as tc:
        with ExitStack() as ctx:
            bc = ctx.enter_context(tc.tile_pool(name="bc", bufs=2))
            tg = ctx.enter_context(tc.tile_pool(name="tg", bufs=4))
            wk = ctx.enter_context(tc.tile_pool(name="wk", bufs=2))
            ot = ctx.enter_context(tc.tile_pool(name="ot", bufs=4))
            ac = ctx.enter_context(tc.tile_pool(name="ac", bufs=1))

            accs = []
            for mb in range(2):
                a = ac.tile([128, N], f32, tag=f"acc{mb}", name=f"acc{mb}")
                nc.gpsimd.memset(a[:], 0.0)
                accs.append(a)

            def wt(tag):
                return wk.tile([128, N], f32, tag=tag, name=tag)

            for b in range(BL):
                BT = bc.tile([128, NQ * N], f32, tag="BT", name="BT")
                nc.sync.dma_start(BT[:], predq[b, :].partition_broadcast(128))

                def Bq(q):
                    return BT[:, q * N:(q + 1) * N]

                for mb in range(2):
                    T = tg.tile([128, NQ], f32, tag="T", name="T")
                    nc.sync.dma_start(T[:], tgtq[b, mb * 128:(mb + 1) * 128, :])

                    def s(q):
                        return T[:, q:q + 1]

                    # intersection
                    ix2 = wt("ix2"); nc.vector.tensor_scalar_min(ix2[:], Bq(2), s(2))
                    ix1 = wt("ix1"); nc.vector.tensor_scalar_max(ix1[:], Bq(0), s(0))
                    dx = wt("dx"); nc.vector.tensor_sub(dx[:], ix2[:], ix1[:])
                    iy2 = wt("iy2"); nc.gpsimd.tensor_scalar_min(iy2[:], Bq(3), s(3))
                    iy1 = wt("iy1"); nc.gpsimd.tensor_scalar_max(iy1[:], Bq(1), s(1))
                    dy = wt("dy"); nc.gpsimd.tensor_sub(dy[:], iy2[:], iy1[:])
                    rx = wt("rx"); nc.scalar.activation(rx[:], dx[:], AF.Relu)
                    ry = wt("ry"); nc.scalar.activation(ry[:], dy[:], AF.Relu)
                    inter = wt("inter"); nc.vector.tensor_mul(inter[:], rx[:], ry[:])

                    # iou_e = inter / (area_p + (area_t + eps) - inter)
                    u1 = wt("u1"); nc.gpsimd.tensor_scalar_add(u1[:], Bq(4), s(4))
                    une = wt("une"); nc.vector.tensor_sub(une[:], u1[:], inter[:])
                    rcpu = wt("rcpu"); nc.vector.reciprocal(rcpu[:], une[:])
                    qq = wt("qq"); nc.vector.tensor_mul(qq[:], inter[:], rcpu[:])

                    # enclosing-box diagonal
                    ex2t = wt("ex2t"); nc.vector.tensor_scalar_max(ex2t[:], Bq(2), s(2))
                    ex1t = wt("ex1t"); nc.vector.tensor_scalar_min(ex1t[:], Bq(0), s(0))
                    ex = wt("ex"); nc.vector.tensor_sub(ex[:], ex2t[:], ex1t[:])
                    ey2t = wt("ey2t"); nc.gpsimd.tensor_scalar_max(ey2t[:], Bq(3), s(3))
                    ey1t = wt("ey1t"); nc.gpsimd.tensor_scalar_min(ey1t[:], Bq(1), s(1))
                    ey = wt("ey"); nc.gpsimd.tensor_sub(ey[:], ey2t[:], ey1t[:])
                    ex2 = wt("ex2"); nc.scalar.activation(ex2[:], ex[:], AF.Square)
                    ey2 = wt("ey2"); nc.scalar.activation(ey2[:], ey[:], AF.Square)
                    diag = wt("diag"); nc.vector.tensor_add(diag[:], ex2[:], ey2[:])
                    diag_e = wt("diag_e")
                    nc.gpsimd.tensor_scalar_add(diag_e[:], diag[:], float(EPS32))
                    rcpd = wt("rcpd"); nc.vector.reciprocal(rcpd[:], diag_e[:])

                    # center distance: (0.5*sx_p - 0.5*sx_t)^2 + ...
                    dcx2 = wt("dcx2")
                    nc.scalar.activation(dcx2[:], Bq(5), AF.Square, bias=s(5), scale=0.5)
                    dcy2 = wt("dcy2")
                    nc.scalar.activation(dcy2[:], Bq(6), AF.Square, bias=s(6), scale=0.5)
                    dist = wt("dist"); nc.gpsimd.tensor_add(dist[:], dcx2[:], dcy2[:])
                    dd = wt("dd"); nc.vector.tensor_mul(dd[:], dist[:], rcpd[:])

                    out = ot.tile([128, N], f32, tag="out", name="out")
                    nc.vector.tensor_sub(out[:], dd[:], qq[:])
                    nc.sync.dma_start(slab[b, mb * 128:(mb + 1) * 128, :], out[:])

                    # L1 partial: acc += sum_c |pred_c - tgt_c| (raw cxcywh)
                    d0 = wt("d0")
                    nc.scalar.activation(d0[:], Bq(7), AF.Abs, bias=s(7))
                    d1 = wt("d1")
                    nc.scalar.activation(d1[:], Bq(8), AF.Abs, bias=s(8))
                    d2 = wt("d2")
                    nc.scalar.activation(d2[:], Bq(9), AF.Abs, bias=s(9))
                    d3 = wt("d3")
                    nc.scalar.activation(d3[:], Bq(10), AF.Abs, bias=s(10))
                    t1 = wt("t1"); nc.vector.tensor_add(t1[:], d0[:], d1[:])
                    t2 = wt("t2"); nc.gpsimd.tensor_add(t2[:], d2[:], d3[:])
                    t3 = wt("t3"); nc.vector.tensor_add(t3[:], t1[:], t2[:])
                    nc.vector.tensor_add(accs[mb][:], accs[mb][:], t3[:])

            nc.sync.dma_start(l1p[0:128, :], accs[0][:])
            nc.sync.dma_start(l1p[128:256, :], accs[1][:])

    nc.compile()
    return nc


def _get_bass():
    global _bass_module
    if _bass_module is None:
        _bass_module = _build_bass()
    return _bass_module


def _preprocess(bbox_pred, bbox_target):
    """Host-side per-box quantities, f32 ops mirroring the reference."""
    f32 = np.float32
    bp = np.asarray(bbox_pred, dtype=f32)
    bt = np.asarray(bbox_target, dtype=f32)
    cx, cy, w, h = bp[..., 0], bp[..., 1], bp[..., 2], bp[..., 3]
    px1 = cx - w / 2; px2 = cx + w / 2
    py1 = cy - h / 2; py2 = cy + h / 2
    parea = (px2 - px1) * (py2 - py1)
    psx = px1 + px2; psy = py1 + py2
    # predq[b, q, n] with q: x1,y1,x2,y2,area,sx,sy,cx,cy,w,h
    predq = np.stack([px1, py1, px2, py2, parea, psx, psy, cx, cy, w, h],
                     axis=1).astype(f32)                       # [B, NQ, N]
    gx, gy, gw, gh = bt[..., 0], bt[..., 1], bt[..., 2], bt[..., 3]
    tx1 = gx - gw / 2; tx2 = gx + gw / 2
    ty1 = gy - gh / 2; ty2 = gy + gh / 2
    tarea_eps = (tx2 - tx1) * (ty2 - ty1) + EPS32
    nsx = f32(-0.5) * (tx1 + tx2); nsy = f32(-0.5) * (ty1 + ty2)
    tgtq = np.stack([tx1, ty1, tx2, ty2, tarea_eps, nsx, nsy,
                     -gx, -gy, -gw, -gh], axis=2).astype(f32)  # [B, M, NQ]
    return predq.reshape(B, NQ * N), tgtq


def _label_cost_T(labels_pred, labels_target):
    """lcT[m, n] = mean_b bce(p[b,n], t[b,m]); f32 elementwise like jax."""
    f32 = np.float32
    x = np.asarray(labels_pred, dtype=f32)[..., 0]
    p = (f32(1.0) / (f32(1.0) + np.exp(-x))).astype(f32)
    lnp = np.maximum(np.log(p), f32(-100.0)).astype(f32)
    ln1 = np.maximum(np.log((f32(1.0) - p).astype(f32)), f32(-100.0)).astype(f32)
    t = np.asarray(labels_target, dtype=np.float64)            # [B, M]
    a = lnp.astype(np.float64); c = ln1.astype(np.float64)     # [B, N]
    return -(t.T @ a + (1.0 - t.T) @ c) / B                    # [M, N] f64


def _solve_assignments(costT):
    """costT: [B, M, N] f64. Returns cols[b, n] = matched target index."""
    from scipy.optimize import linear_sum_assignment
    cols = np.empty((B, N), dtype=np.int64)

    def solve(b):
        row_ind, col_ind = linear_sum_assignment(costT[b])
        cols[b, col_ind] = row_ind

    try:
        from concurrent.futures import ThreadPoolExecutor
        with ThreadPoolExecutor(max_workers=8) as tp:
            list(tp.map(solve, range(B)))
    except Exception:
        for b in range(B):
            solve(b)
    return cols


def _final_losses(labels_pred, bbox_pred, labels_target, bbox_target, cols):
    f64 = np.float64
    bp = np.asarray(bbox_pred, dtype=f64)
    bt = np.asarray(bbox_target, dtype=f64)
    lt = np.asarray(labels_target, dtype=f64)
    x = np.asarray(labels_pred, dtype=np.float32)[..., 0]
    p32 = (np.float32(1.0) / (np.float32(1.0) + np.exp(-x))).astype(np.float32)
    p = p32.astype(f64)

    bi = np.arange(B)[:, None]
    t_m = lt[bi, cols]
    bt_m = bt[bi, cols]
    wm = (t_m == 1.0).astype(f64)

    def xyxy(bb):
        c_x, c_y, ww, hh = bb[..., 0], bb[..., 1], bb[..., 2], bb[..., 3]
        return (c_x - ww / 2, c_y - hh / 2, c_x + ww / 2, c_y + hh / 2)

    x1, y1, x2, y2 = xyxy(bp)
    xg1, yg1, xg2, yg2 = xyxy(bt_m)
    xi1 = np.maximum(x1, xg1); yi1 = np.maximum(y1, yg1)
    xi2 = np.minimum(x2, xg2); yi2 = np.minimum(y2, yg2)
    inter = np.clip(xi2 - xi1, 0, None) * np.clip(yi2 - yi1, 0, None)
    union = (x2 - x1) * (y2 - y1) + (xg2 - xg1) * (yg2 - yg1) - inter
    iou_p = inter / union
    iou_e = inter / (union + 1e-7)
    xc1 = np.minimum(x1, xg1); yc1 = np.minimum(y1, yg1)
    xc2 = np.maximum(x2, xg2); yc2 = np.maximum(y2, yg2)
    diag = (xc2 - xc1) ** 2 + (yc2 - yc1) ** 2 + 1e-7
    dist = ((x1 + x2 - xg1 - xg2) * 0.5) ** 2 + ((y1 + y2 - yg1 - yg2) * 0.5) ** 2
    diou_e = 1.0 - iou_e + dist / diag

    wsum = wm.sum()
    diou_loss = (diou_e * wm).sum() / wsum
    iou_out = (iou_p * wm).sum() / wsum
    lnp = np.maximum(np.log(p), -100.0)
    ln1 = np.maximum(np.log1p(-p), -100.0)
    label_loss = (-(t_m * lnp + (1.0 - t_m) * ln1)).mean()
    bbox_loss = (np.abs(bp - bt_m) * wm[..., None]).sum() / (wsum * 4.0)
    return diou_loss + label_loss + bbox_loss, iou_out


def kernel(labels_pred, bbox_pred, labels_target, bbox_target):
    from concourse import bass_utils

    nc = _get_bass()
    predq, tgtq = _preprocess(bbox_pred, bbox_target)

    in_maps = [
        {"predq": np.ascontiguousarray(predq[c * BL:(c + 1) * BL]),
         "tgtq": np.ascontiguousarray(tgtq[c * BL:(c + 1) * BL])}
        for c in range(NCORES)
    ]
    res = bass_utils.run_bass_kernel_spmd(nc, in_maps, core_ids=list(range(NCORES)))

    slabT = np.concatenate([res.results[c]["slab"] for c in range(NCORES)],
                           axis=0).astype(np.float64)          # [B, M, N]
    l1T = sum(res.results[c]["l1p"].astype(np.float64)
              for c in range(NCORES)) / (B * 4.0)              # [M, N]
    lcT = _label_cost_T(labels_pred, labels_target)            # [M, N]

    costT = slabT + (l1T + lcT + 1.0)[None, :, :]
    cols = _solve_assignments(costT)

    total, iou = _final_losses(labels_pred, bbox_pred, labels_target,
                               bbox_target, cols)
    return np.float32(total), np.float32(iou)


# revision 9
# speedup vs baseline: 3.2802x; 3.2802x over previous
"""Trainium2 Bass kernel for nn_Loss_fn_21852793602431 (DETR-style loss).

Strategy (data-parallel over batch B=64, 8 cores x 8 batches):
  - host: preprocess per-box quantities (f32, mirroring the reference
    formula order) into per-core device inputs
  - device (SPMD x8): build the transposed DIoU pair-cost slab
    slabT[b, m, n] = dist/diag - iou_e  for the core's 8 batches, plus the
    core-partial L1 cost matrix sum_{b_loc,c} |pred - tgt| (transposed)
  - host: label (BCE) cost via matmul, sum the 8 L1 partials, add, run
    scipy linear_sum_assignment per batch, compute the final scalar losses

Device kernel layout: partitions = m (two 128-blocks), free = (b_page, n)
mega-tiles [128, 8, 256]. Four-input pairwise terms (intersection width,
enclosing-box width) use runtime-registered custom DVE ops with per-partition
target scalars, written page-by-page; two-tensor combining steps run as
whole-mega-tile ops split across DVE/GPSIMD; center-distance squares run on
the scalar engine (Square activation with per-partition bias); the L1 cost
uses |pred+(-tgt)| tiles (DVE custom / ACT Abs) contracted over (batch,coord)
by TensorE matmuls with a 0/1 selector into PSUM.
"""

import sys

if "/opt/trn_rl_repo" not in sys.path:
    sys.path.insert(0, "/opt/trn_rl_repo")

import numpy as np

B, N, M = 64, 256, 256
NCORES = 8
BL = B // NCORES
NQ = 7
EPS32 = np.float32(1e-7)

CFG = {
    "nr": True,          # Newton-refine the fast reciprocals (2 ULP vs 51 ULP)
    "l1_dve_chunks": 3,  # of 8 chunks (8 m-groups each): this many on DVE
    "qdd_engine": "gps", # q = inter*rcp_u, dd = dist*rcp_d
    "out_engine": "gps", # out = dd - q
}

_bass_module = None
_custom_ops = None


def _register_custom_ops():
    global _custom_ops
    if _custom_ops is not None:
        return _custom_ops
    from concourse.dve_ops import (DveOp, OPS, CUSTOM_DVE_SPECS,
                                   _SUB_OPCODE_FOR_NAME, _CUSTOM_DVE_ROW_BASE)
    from concourse.dve_spec import (Spec, Src0, Src1, C0, C1, C2, Zero,
                                    relu, sq, maxx, minn, lower, _has_src1)
    from concourse.dve_uop import DveOpSpec

    existing = {op.name: op for op in OPS}

    def reg(name, body, reference):
        if name in existing:
            return existing[name]
        row = _CUSTOM_DVE_ROW_BASE + len(OPS)
        assert row < 0x20, "custom DVE opcode rows exhausted"
        sha = {}
        for ver in ("v3", "v4"):
            s = DveOpSpec(name=name, opcode=row,
                          uops=lower(Spec(body=body), ver=ver),
                          rd1_en=_has_src1(Spec(body=body)))
            sha[ver] = s.sha(ver)
        op = DveOp(name, Spec(body=body, reference=reference),
                   subdim=False, uops_sha=sha)
        OPS.append(op)
        _SUB_OPCODE_FOR_NAME[name] = row
        CUSTOM_DVE_SPECS[name] = op.spec
        return op

    eps = float(EPS32)
    _custom_ops = {
        # dx = min(x2p, x2t) - max(x1p, x1t)
        "DX": reg("ANT_DX", minn(Src0, C0) - maxx(Src1, C1),
                  lambda in0, in1, s0, s1, imm2:
                  np.minimum(in0, s0) - np.maximum(in1, s1)),
        # inter = relu(dx) * relu(dy)
        "MULRELU": reg("ANT_MULRELU", relu(Src0) * relu(Src1),
                       lambda in0, in1, s0, s1, imm2:
                       np.maximum(in0, 0) * np.maximum(in1, 0)),
        # ex2 = (max(x2p, x2t) - min(x1p, x1t))^2
        "ENCSQ": reg("ANT_ENCSQ", sq(maxx(Src0, C0) - minn(Src1, C1)),
                     lambda in0, in1, s0, s1, imm2:
                     (np.maximum(in0, s0) - np.minimum(in1, s1)) ** 2),
        # ey2e = (max(y2p, y2t) - min(y1p, y1t))^2 + imm2 (eps)
        "ENCSQE": reg("ANT_ENCSQE",
                      sq(maxx(Src0, C0) - minn(Src1, C1)) + C2,
                      lambda in0, in1, s0, s1, imm2:
                      (np.maximum(in0, s0) - np.minimum(in1, s1)) ** 2
                      + np.float32(imm2)),
        # l1 term: |S0 + S1|  (S1 carries -tgt, broadcast along n)
        "ABSADD": reg("ANT_ABSADD",
                      maxx(Src0 + Src1, Zero - (Src0 + Src1)),
                      lambda in0, in1, s0, s1, imm2: np.abs(in0 + in1)),
    }
    return _custom_ops


def _build_bass():
    import concourse.bacc as bacc
    from concourse import mybir, tile
    from contextlib import ExitStack

    ops = _register_custom_ops()
    f32 = mybir.dt.float32
    AF = mybir.ActivationFunctionType

    nc = bacc.Bacc("TRN2", target_bir_lowering=False, debug=False,
                   num_devices=NCORES)
    predq = nc.dram_tensor("predq", [BL, NQ, N], f32, kind="ExternalInput").ap()
    tgtq = nc.dram_tensor("tgtq", [BL, M, NQ], f32, kind="ExternalInput").ap()
    predl1 = nc.dram_tensor("predl1", [128, N], f32, kind="ExternalInput").ap()
    tgtl1n = nc.dram_tensor("tgtl1n", [128, 64], f32, kind="ExternalInput").ap()
    selb = nc.dram_tensor("selb", [128, 256], f32, kind="ExternalInput").ap()
    slab = nc.dram_tensor("slab", [BL, M, N], f32, kind="ExternalOutput").ap()
    l1p = nc.dram_tensor("l1p", [M, N], f32, kind="ExternalOutput").ap()

    vec, gps, act = nc.vector, nc.gpsimd, nc.scalar
    qdd = gps if CFG["qdd_engine"] == "gps" else vec
    oute = gps if CFG["out_engine"] == "gps" else vec

    with tile.TileContext(nc) as tc:
        with ExitStack() as ctx:
            pb = ctx.enter_context(tc.tile_pool(name="pb", bufs=1))
            tg = ctx.enter_context(tc.tile_pool(name="tg", bufs=2))
            wk = ctx.enter_context(tc.tile_pool(name="wk", bufs=1))
            ot = ctx.enter_context(tc.tile_pool(name="ot", bufs=2))
            l1w = ctx.enter_context(tc.tile_pool(name="l1w", bufs=2))
            cst = ctx.enter_context(tc.tile_pool(name="cst", bufs=1))
            psp = ctx.enter_context(tc.tile_pool(name="psp", bufs=1, space="PSUM"))

            # ---- L1 cost: |pred + (-tgt)| contracted over (b, c) by PE ----
            pl1 = cst.tile([128, 8, N], f32, name="pl1")
            nc.sync.dma_start(pl1[:], predl1[:, None, :].broadcast_to((128, 8, N)))
            tl1 = cst.tile([128, 64], f32, name="tl1")
            nc.sync.dma_start(tl1[:], tgtl1n)
            o4 = cst.tile([128, 256], f32, name="o4")
            nc.sync.dma_start(o4[:], selb)
            pss = [psp.tile([128, N], f32, name=f"ps{i}") for i in range(2)]

            nchunk_dve = CFG["l1_dve_chunks"]
            for chunk in range(8):           # 8 m-groups of 4 per chunk
                d = l1w.tile([128, 8, N], f32, tag="d", name="d")
                if chunk < nchunk_dve:
                    tb = tl1[:, chunk * 8:(chunk + 1) * 8]
                    vec._custom_dve(ops["ABSADD"], out=d[:],
                                    in0=pl1[:],
                                    in1=tb[:, :, None].broadcast_to((128, 8, N)))
                else:
                    for j in range(8):
                        g = chunk * 8 + j
                        act.activation(d[:, j, :], pl1[:, j, :], AF.Abs,
                                       bias=tl1[:, g:g + 1])
                for j in range(8):
                    g = chunk * 8 + j
                    mb, loc = g // 32, g % 32
                    nc.tensor.matmul(pss[mb][:],
                                     o4[:, 124 - 4 * loc:252 - 4 * loc],
                                     d[:, j, :],
                                     start=(loc == 0), stop=(loc == 31))
            for mb in range(2):
                l1sb = cst.tile([128, N], f32, tag="l1sb", name=f"l1sb{mb}")
                nc.scalar.copy(l1sb[:], pss[mb][:])
                nc.sync.dma_start(l1p[mb * 128:(mb + 1) * 128, :], l1sb[:])

            # ---- pred broadcast mega-tiles [128, 8, 256] ----
            PB = []
            for q in range(NQ):
                t = pb.tile([128, BL, N], f32, name=f"pb{q}")
                nc.sync.dma_start(t[:], predq[:, q, :].partition_broadcast(128))
                PB.append(t)

            def wt(tag):
                return wk.tile([128, BL, N], f32, tag=tag, name=tag)

            for mb in range(2):
                T = tg.tile([128, BL, NQ], f32, tag="T", name="T")
                nc.sync.dma_start(
                    T[:], tgtq[:, mb * 128:(mb + 1) * 128, :]
                    .rearrange("b m q -> m b q"))

                def bc(qi):
                    return T[:, :, qi][:, :, None].broadcast_to((128, BL, N))

                dx = wt("dx"); dy = wt("dy")
                ex2 = wt("ex2"); ey2e = wt("ey2e")
                dcx = wt("dcx"); dcy = wt("dcy")
                for b in range(BL):
                    s = lambda qi: T[:, b, qi:qi + 1]
                    vec._custom_dve(ops["DX"], out=dx[:, b, :],
                                    in0=PB[2][:, b, :], in1=PB[0][:, b, :],
                                    s0=s(0 + 2), s1=s(0))
                    vec._custom_dve(ops["DX"], out=dy[:, b, :],
                                    in0=PB[3][:, b, :], in1=PB[1][:, b, :],
                                    s0=s(3), s1=s(1))
                    vec._custom_dve(ops["ENCSQ"], out=ex2[:, b, :],
                                    in0=PB[2][:, b, :], in1=PB[0][:, b, :],
                                    s0=s(2), s1=s(0))
                    vec._custom_dve(ops["ENCSQE"], out=ey2e[:, b, :],
                                    in0=PB[3][:, b, :], in1=PB[1][:, b, :],
                                    s0=s(3), s1=s(1), imm2=float(EPS32))
                    # (0.5*sx_p + (-0.5*sx_t))^2
                    act.activation(dcx[:, b, :], PB[5][:, b, :], AF.Square,
                                   bias=s(5), scale=0.5)
                    act.activation(dcy[:, b, :], PB[6][:, b, :], AF.Square,
                                   bias=s(6), scale=0.5)

                inter = wt("inter")
                vec._custom_dve(ops["MULRELU"], out=inter[:], in0=dx[:], in1=dy[:])
                u1 = wt("u1")
                gps.tensor_add(u1[:], PB[4][:], bc(4))       # area_p + area_t'
                gps.tensor_sub(u1[:], u1[:], inter[:])       # une
                gps.tensor_add(ex2[:], ex2[:], ey2e[:])      # diag_e
                gps.tensor_add(dcx[:], dcx[:], dcy[:])       # dist

                rfu = wt("rfu"); rfd = wt("rfd")
                vec.reciprocal_approx_fast(rfu[:], u1[:])
                vec.reciprocal_approx_fast(rfd[:], ex2[:])
                if CFG["nr"]:
                    from concourse.dve_ops import RECIPROCAL_APPROX_NR
                    vec._custom_dve(RECIPROCAL_APPROX_NR, out=rfu[:],
                                    in0=u1[:], in1=rfu[:], s0=2.0)
                    vec._custom_dve(RECIPROCAL_APPROX_NR, out=rfd[:],
                                    in0=ex2[:], in1=rfd[:], s0=2.0)
                qdd.tensor_mul(inter[:], inter[:], rfu[:])   # q = iou_e
                qdd.tensor_mul(dcx[:], dcx[:], rfd[:])       # dd = dist/diag
                outm = ot.tile([128, BL, N], f32, tag="outm", name="outm")
                oute.tensor_sub(outm[:], dcx[:], inter[:])
                nc.sync.dma_start(
                    slab[:, mb * 128:(mb + 1) * 128, :]
                    .rearrange("b m n -> m b n"), outm[:])

    nc.compile()
    return nc


def _get_bass():
    global _bass_module
    if _bass_module is None:
        _bass_module = _build_bass()
    return _bass_module


def _preprocess(bbox_pred, bbox_target):
    """Host-side per-box quantities, f32 ops mirroring the reference."""
    f32 = np.float32
    bp = np.asarray(bbox_pred, dtype=f32)
    bt = np.asarray(bbox_target, dtype=f32)
    cx, cy, w, h = bp[..., 0], bp[..., 1], bp[..., 2], bp[..., 3]
    px1 = cx - w / 2; px2 = cx + w / 2
    py1 = cy - h / 2; py2 = cy + h / 2
    parea = (px2 - px1) * (py2 - py1)
    psx = px1 + px2; psy = py1 + py2
    predq = np.stack([px1, py1, px2, py2, parea, psx, psy],
                     axis=1).astype(f32)                       # [B, NQ, N]
    gx, gy, gw, gh = bt[..., 0], bt[..., 1], bt[..., 2], bt[..., 3]
    tx1 = gx - gw / 2; tx2 = gx + gw / 2
    ty1 = gy - gh / 2; ty2 = gy + gh / 2
    tarea_eps = (tx2 - tx1) * (ty2 - ty1) + EPS32
    nsx = f32(-0.5) * (tx1 + tx2); nsy = f32(-0.5) * (ty1 + ty2)
    tgtq = np.stack([tx1, ty1, tx2, ty2, tarea_eps, nsx, nsy],
                    axis=2).astype(f32)                        # [B, M, NQ]

    # L1 inputs per core: partition j = rep*32 + b_loc*4 + c
    rep = np.arange(128) // 32
    bj = (np.arange(128) % 32) // 4
    cj = np.arange(128) % 4
    predl1 = np.empty((NCORES, 128, N), dtype=f32)
    tgtl1n = np.empty((NCORES, 128, 64), dtype=f32)
    g = np.arange(64)
    mm = 4 * g[None, :] + rep[:, None]                         # [128, 64]
    for core in range(NCORES):
        bg = core * BL + bj
        predl1[core] = bp[bg, :, cj]
        tgtl1n[core] = -bt[bg[:, None], mm, cj[:, None]]
    selb = (np.arange(256)[None, :] == 124 + rep[:, None]).astype(f32)
    return predq, tgtq, predl1, tgtl1n, selb


def _label_cost_T(labels_pred, labels_target):
    """lcT[m, n] = mean_b bce(p[b,n], t[b,m]); f32 elementwise like jax."""
    f32 = np.float32
    x = np.asarray(labels_pred, dtype=f32)[..., 0]
    p = (f32(1.0) / (f32(1.0) + np.exp(-x))).astype(f32)
    lnp = np.maximum(np.log(p), f32(-100.0)).astype(f32)
    ln1 = np.maximum(np.log((f32(1.0) - p).astype(f32)), f32(-100.0)).astype(f32)
    t = np.asarray(labels_target, dtype=np.float64)            # [B, M]
    a = lnp.astype(np.float64); c = ln1.astype(np.float64)     # [B, N]
    return -(t.T @ a + (1.0 - t.T) @ c) / B                    # [M, N] f64


def _solve_assignments(costT):
    """costT: [B, M, N] f64. Returns cols[b, n] = matched target index."""
    from scipy.optimize import linear_sum_assignment
    cols = np.empty((B, N), dtype=np.int64)

    def solve(b):
        row_ind, col_ind = linear_sum_assignment(costT[b])
        cols[b, col_ind] = row_ind

    try:
        from concurrent.futures import ThreadPoolExecutor
        with ThreadPoolExecutor(max_workers=8) as tp:
            list(tp.map(solve, range(B)))
    except Exception:
        for b in range(B):
            solve(b)
    return cols


def _final_losses(labels_pred, bbox_pred, labels_target, bbox_target, cols):
    f64 = np.float64
    bp = np.asarray(bbox_pred, dtype=f64)
    bt = np.asarray(bbox_target, dtype=f64)
    lt = np.asarray(labels_target, dtype=f64)
    x = np.asarray(labels_pred, dtype=np.float32)[..., 0]
    p32 = (np.float32(1.0) / (np.float32(1.0) + np.exp(-x))).astype(np.float32)
    p = p32.astype(f64)

    bi = np.arange(B)[:, None]
    t_m = lt[bi, cols]
    bt_m = bt[bi, cols]
    wm = (t_m == 1.0).astype(f64)

    def xyxy(bb):
        c_x, c_y, ww, hh = bb[..., 0], bb[..., 1], bb[..., 2], bb[..., 3]
        return (c_x - ww / 2, c_y - hh / 2, c_x + ww / 2, c_y + hh / 2)

    x1, y1, x2, y2 = xyxy(bp)
    xg1, yg1, xg2, yg2 = xyxy(bt_m)
    xi1 = np.maximum(x1, xg1); yi1 = np.maximum(y1, yg1)
    xi2 = np.minimum(x2, xg2); yi2 = np.minimum(y2, yg2)
    inter = np.clip(xi2 - xi1, 0, None) * np.clip(yi2 - yi1, 0, None)
    union = (x2 - x1) * (y2 - y1) + (xg2 - xg1) * (yg2 - yg1) - inter
    iou_p = inter / union
    iou_e = inter / (union + 1e-7)
    xc1 = np.minimum(x1, xg1); yc1 = np.minimum(y1, yg1)
    xc2 = np.maximum(x2, xg2); yc2 = np.maximum(y2, yg2)
    diag = (xc2 - xc1) ** 2 + (yc2 - yc1) ** 2 + 1e-7
    dist = ((x1 + x2 - xg1 - xg2) * 0.5) ** 2 + ((y1 + y2 - yg1 - yg2) * 0.5) ** 2
    diou_e = 1.0 - iou_e + dist / diag

    wsum = wm.sum()
    diou_loss = (diou_e * wm).sum() / wsum
    iou_out = (iou_p * wm).sum() / wsum
    lnp = np.maximum(np.log(p), -100.0)
    ln1 = np.maximum(np.log1p(-p), -100.0)
    label_loss = (-(t_m * lnp + (1.0 - t_m) * ln1)).mean()
    bbox_loss = (np.abs(bp - bt_m) * wm[..., None]).sum() / (wsum * 4.0)
    return diou_loss + label_loss + bbox_loss, iou_out


def kernel(labels_pred, bbox_pred, labels_target, bbox_target):
    from concourse import bass_utils

    nc = _get_bass()
    predq, tgtq, predl1, tgtl1n, selb = _preprocess(bbox_pred, bbox_target)

    in_maps = [
        {"predq": np.ascontiguousarray(predq[c * BL:(c + 1) * BL]),
         "tgtq": np.ascontiguousarray(tgtq[c * BL:(c + 1) * BL]),
         "predl1": np.ascontiguousarray(predl1[c]),
         "tgtl1n": np.ascontiguousarray(tgtl1n[c]),
         "selb": selb}
        for c in range(NCORES)
    ]
    res = bass_utils.run_bass_kernel_spmd(nc, in_maps, core_ids=list(range(NCORES)))

    slabT = np.concatenate([res.results[c]["slab"] for c in range(NCORES)],
                           axis=0).astype(np.float64)          # [B, M, N]
    l1T = sum(res.results[c]["l1p"].astype(np.float64)
              for c in range(NCORES)) / (B * 4.0)              # [M, N]
    lcT = _label_cost_T(labels_pred, labels_target)            # [M, N]

    costT = slabT + (l1T + lcT + 1.0)[None, :, :]
    cols = _solve_assignments(costT)

    total, iou = _final_losses(labels_pred, bbox_pred, labels_target,
                               bbox_target, cols)
    return np.float32(total), np.float32(iou)
